# revision 2
# baseline (speedup 1.0000x reference)
"""Trainium2 Bass kernel for nn_LiquidNeuralNetwork.

Strategy: data-parallel over batch (8 cores x 64 batch). Per core, a fully
on-chip recurrence over T=512 steps. Layers are software-pipelined one time
step apart (layer0 at t, layer1 at t-1) and packed side-by-side in the free
dim so elementwise/activation ops cover both layers in one instruction.

Layout: compute tiles are [128 partitions = h%128, free = l*128 + q*64 + b]
(l = layer stream, q = H half, b = batch-in-core). All linear algebra of the
LTC cell (input projections, recurrent matmuls, -h/tau diagonal terms, RK4
hh-increments) is accumulated in PSUM by the tensor engine using bf16
weights; only the gate nonlinearity (tanh, sigmoid on ScalarE), the gated
product and the RK4 k-assembly run on the vector engines. LayerNorm is done
with ones-matmul partition reductions, a Newton-iteration rsqrt on the DVE
(no sqrt activation table thrash), and rank-1/rank-2 matmul broadcasts that
fold gamma/beta/mu/rstd into a single multiply-add before the output tanh.
"""

import os
import sys
import numpy as np

sys.path.insert(0, "/opt/trn_rl_repo")

B, T, FS, FC, H, L = 512, 512, 64, 32, 256, 2
LN_EPS = 1e-5
NCORES = 8
BC = B // NCORES       # 64 batch per core
NF = 2 * H * BC // 128  # 256 free size of packed tiles
UNROLL = 8
# Newton rsqrt seed: y0 = 1/(a + b*vv), vv = N^2*(var+36eps), guaranteed
# underestimate when 4ab >= 1. Calibrated for vv in [VLO, VHI].
N_H = 256.0
RSQRT_ITERS = 3


def _seed_consts(vlo, vhi):
    vc = float(np.sqrt(vlo * vhi))
    b = 0.5 / np.sqrt(vc)
    a = 1.0 / (4.0 * b)
    return a, b


# vv = N^2 * (var(P) + 36eps); var(P) observed ~[1, 400] (P = 6*hn_pre), with
# big safety margin -> vv range [256^2*0.05, 256^2*4000]
SEED_A, SEED_B = _seed_consts(256.0 * 256.0 * 0.05, 256.0 * 256.0 * 4000.0)


def softplus_np(x):
    return np.log1p(np.exp(-np.abs(x))) + np.maximum(x, 0)


def _bf16(x):
    import ml_dtypes
    return np.asarray(x, np.float32).astype(ml_dtypes.bfloat16)


# ---------------------------------------------------------------------------
# Bass module builder
# ---------------------------------------------------------------------------

def build_module(T_run=T, unroll=UNROLL):
    import concourse.bass as bass
    import concourse.mybir as mybir
    from concourse import tile, bacc
    from concourse.bass import ds

    f32 = mybir.dt.float32
    bf16 = mybir.dt.bfloat16
    AF = mybir.ActivationFunctionType
    OP = mybir.AluOpType

    nc = bacc.Bacc(None, target_bir_lowering=False)

    # ---- DRAM I/O -------------------------------------------------------
    xcat = nc.declare_dram_parameter("xcat", [T_run, 97, BC], bf16, isOutput=False)
    # weights, pre-transposed/packed on host (see _prep_weights)
    wdecl = {}

    def wparam(name, shape, dtype=bf16):
        wdecl[name] = nc.declare_dram_parameter(name, shape, dtype, isOutput=False)
        return wdecl[name]

    # layer0: K-dims: x-part 97 (seq 64 + ctx 32 + bias 1), h-part 256
    wparam("g_x0", [97, 256])           # [Wg0_x.T ; bg0] , M=(q*128)
    wparam("in_x0", [96, 256])          # Win0_cat.T
    wparam("g_h0", [128, 512])          # (kt, m*128): Wg0_h.T tiles
    wparam("g_h0h", [128, 512])         # x0.5
    wparam("rec0", [128, 512])
    wparam("rec0h", [128, 512])
    wparam("dneg0", [128, 256])         # per q: diag(-itau0)
    wparam("dneg0h", [128, 256])        # per q: diag(-0.5*itau0)
    # layer1: x-part = h0 (K=256), h-part 256
    wparam("g_x1", [128, 512])
    wparam("bg1row", [1, 256])          # bias row, M=(q*128)
    wparam("in_x1", [128, 512])
    wparam("g_h1", [128, 512])
    wparam("g_h1h", [128, 512])
    wparam("rec1", [128, 512])
    wparam("rec1h", [128, 512])
    wparam("dneg1", [128, 256])
    wparam("dneg1h", [128, 256])
    # P-build identity variants
    wparam("id6", [128, 128])
    wparam("id1", [128, 128])
    wparam("id2", [128, 128])
    # LN: per (l,q): row0=lnb, row1=-lng ; and lngN rows (lng*N) for s'
    wparam("lnT", [2, 512])             # [(l*2+q)*128]
    wparam("lngN", [1, 512])
    wparam("ones_red", [128, 1])
    wparam("ones2", [2, 128])
    hout = nc.declare_dram_parameter("hout", [128, 128], bf16, isOutput=True)

    NITER = (T_run - 1 - ((T_run - 1) % unroll)) // unroll  # loop covers ticks 1..NITER*unroll
    loop_hi = 1 + NITER * unroll

    with tile.TileContext(nc) as tc:
        from contextlib import ExitStack
        with ExitStack() as ctx:
            singles = ctx.enter_context(tc.tile_pool(name="singles", bufs=1))
            xc_pool = ctx.enter_context(tc.tile_pool(name="xc", bufs=3))
            g_pool = ctx.enter_context(tc.tile_pool(name="g", bufs=2))
            k_pool = ctx.enter_context(tc.tile_pool(name="k", bufs=3))
            s_pool = ctx.enter_context(tc.tile_pool(name="s", bufs=3))
            row_pool = ctx.enter_context(tc.tile_pool(name="row", bufs=4))
            g_psum = ctx.enter_context(tc.tile_pool(name="gps", bufs=1, space="PSUM"))
            r_psum = ctx.enter_context(tc.tile_pool(name="rps", bufs=1, space="PSUM"))
            v_psum = ctx.enter_context(tc.tile_pool(name="vps", bufs=1, space="PSUM"))
            p_psum = ctx.enter_context(tc.tile_pool(name="pps", bufs=1, space="PSUM"))
            st_psum = ctx.enter_context(tc.tile_pool(name="stp", bufs=2, space="PSUM"))
            bc_psum = ctx.enter_context(tc.tile_pool(name="bcp", bufs=2, space="PSUM"))

            # ---- load weights to SBUF (resident) -------------------------
            W = {}
            for name, dram in wdecl.items():
                t_ = singles.tile(list(dram.shape), dram.dtype, name=name, tag=name)
                nc.sync.dma_start(t_[:], dram[:])
                W[name] = t_

            # ring of state tiles; slot(tau) = tau % 8 holds output of tick tau
            ring = [singles.tile([128, 256], bf16, name=f"ring{i}", tag=f"ring{i}") for i in range(8)]
            hz = singles.tile([128, 256], bf16)   # zero state
            nc.vector.memset(hz[:], 0.0)
            trhs = singles.tile([2, 128], bf16)   # rank-2 rhs; row0=nm, row1=1
            nc.sync.dma_start(trhs[:, :], wdecl["ones2"][:, :])
            onesr = singles.tile([1, 64], bf16)   # ones rhs row for bg1
            nc.vector.memset(onesr[:], 1.0)

            def slot(tau):
                return ring[tau % 8]

            # W slicing helpers: packed (kt, m) tiles
            def wt(name, kt, m):
                return W[name][:, kt * 256 + m * 128: kt * 256 + (m + 1) * 128]

            def dneg(l, half, m):
                nm = f"dneg{l}" + ("h" if half else "")
                return W[nm][:, m * 128:(m + 1) * 128]

            # free-dim column helpers for packed [128, 256] tiles
            def col(l, q):
                return slice(l * 128 + q * 64, l * 128 + (q + 1) * 64)

            def emit_tick(xc, h_prev, h1_prev, h_next, do0, do1):
                """h_prev: state tile holding (h0[t-1] | h1[t-2]-ish l0 half
                used for both L0 state and L1 input; h1_prev: tile whose l1
                half is layer1's state. h_next: output tile."""
                lo = 0 if do0 else 128
                hi = 256 if do1 else 128
                cs = slice(lo, hi)          # full packed range this tick
                l_first = 0 if do0 else 1
                l_last = 1 if do1 else 0
                # -------- per-step accumulation banks (one group each) ---
                G = g_psum.tile([128, 256], f32, tag="G")
                R = r_psum.tile([128, 256], f32, tag="R")
                V = v_psum.tile([128, 256], f32, tag="V")
                _first = {}

                def bmm(bank, out_ap, lhsT, rhs, last=False):
                    st_ = bank not in _first
                    _first[bank] = True
                    nc.tensor.matmul(out_ap, lhsT, rhs, start=st_, stop=last, skip_group_check=True)
                # G/R/V initial (stage-1) content
                if do0:
                    for m in range(2):
                        bmm("G", G[:, col(0, m)], W["g_x0"][:, m * 128:(m + 1) * 128], xc[:, :])
                        bmm("G", G[:, col(0, m)], wt("g_h0", 0, m), h_prev[:, 0:64])
                        bmm("G", G[:, col(0, m)], wt("g_h0", 1, m), h_prev[:, 64:128])
                        bmm("R", R[:, col(0, m)], wt("rec0", 0, m), h_prev[:, 0:64])
                        bmm("R", R[:, col(0, m)], wt("rec0", 1, m), h_prev[:, 64:128])
                        bmm("V", V[:, col(0, m)], W["in_x0"][:, m * 128:(m + 1) * 128], xc[0:96, :])
                        bmm("V", V[:, col(0, m)], dneg(0, False, m), h_prev[:, m * 64:(m + 1) * 64])
                if do1:
                    x1 = h_prev  # l0 half = h0[t-1] = layer1 input
                    for m in range(2):
                        bmm("G", G[:, col(1, m)], wt("g_x1", 0, m), x1[:, 0:64])
                        bmm("G", G[:, col(1, m)], wt("g_x1", 1, m), x1[:, 64:128])
                        bmm("G", G[:, col(1, m)], W["bg1row"][:, m * 128:(m + 1) * 128], onesr[:, :])
                        bmm("G", G[:, col(1, m)], wt("g_h1", 0, m), h1_prev[:, 128:192])
                        bmm("G", G[:, col(1, m)], wt("g_h1", 1, m), h1_prev[:, 192:256])
                        bmm("R", R[:, col(1, m)], wt("rec1", 0, m), h1_prev[:, 128:192])
                        bmm("R", R[:, col(1, m)], wt("rec1", 1, m), h1_prev[:, 192:256])
                        bmm("V", V[:, col(1, m)], wt("in_x1", 0, m), x1[:, 0:64])
                        bmm("V", V[:, col(1, m)], wt("in_x1", 1, m), x1[:, 64:128])
                        bmm("V", V[:, col(1, m)], dneg(1, False, m), h1_prev[:, 128 + m * 64:128 + (m + 1) * 64])

                ks = []
                P_sb = s_pool.tile([128, 256], bf16, tag="P")
                for s in range(4):
                    if s > 0:
                        # Delta rhs and weight-variant selection
                        if s == 1:
                            dr = ks[0]
                            halfw = True
                        elif s == 2:
                            dr = k_pool.tile([128, 256], bf16, tag="d")
                            nc.vector.scalar_tensor_tensor(out=dr[:, cs], in0=ks[0][:, cs], scalar=-1.0, in1=ks[1][:, cs], op0=OP.mult, op1=OP.add)
                            halfw = True
                        else:
                            dr = k_pool.tile([128, 256], bf16, tag="d")
                            # k3 - 0.5*k2 via cody_waite cascade (1 op)
                            nc.vector.scalar_tensor_tensor(out=dr[:, cs], in0=ks[1][:, cs], scalar=-0.5, in1=ks[2][:, cs], op0=OP.mult, op1=OP.add)
                            halfw = False
                        sfx = "h" if halfw else ""
                        for l in range(l_first, l_last + 1):
                            for m in range(2):
                                last = (s == 3) and (l == l_last) and (m == 1)
                                bmm("G", G[:, col(l, m)], wt(f"g_h{l}{sfx}", 0, m), dr[:, l * 128:l * 128 + 64])
                                bmm("G", G[:, col(l, m)], wt(f"g_h{l}{sfx}", 1, m), dr[:, l * 128 + 64:l * 128 + 128], last=last)
                                bmm("R", R[:, col(l, m)], wt(f"rec{l}{sfx}", 0, m), dr[:, l * 128:l * 128 + 64])
                                bmm("R", R[:, col(l, m)], wt(f"rec{l}{sfx}", 1, m), dr[:, l * 128 + 64:l * 128 + 128], last=last)
                                bmm("V", V[:, col(l, m)], dneg(l, halfw, m), dr[:, l * 128 + m * 64:l * 128 + (m + 1) * 64], last=last)
                    # gate and k
                    tg = g_pool.tile([128, 256], f32, tag="tg")
                    nc.scalar.activation(tg[:, cs], G[:, cs], AF.Tanh)
                    gg = g_pool.tile([128, 256], f32, tag="gg")
                    nc.scalar.activation(gg[:, cs], tg[:, cs], AF.Sigmoid)
                    mm = g_pool.tile([128, 256], f32, tag="mm")
                    nc.vector.scalar_tensor_tensor(out=mm[:, cs], in0=gg[:, cs], scalar=0.0, in1=R[:, cs], op0=OP.add, op1=OP.mult)
                    kk = k_pool.tile([128, 256], bf16, tag=f"k{s}")
                    nc.vector.scalar_tensor_tensor(out=kk[:, cs], in0=mm[:, cs], scalar=0.0, in1=V[:, cs], op0=OP.add, op1=OP.add)
                    ks.append(kk)

                # -------- P = 6h + k1 + 2k2 + 2k3 + k4 (PSUM, PE) --------
                PP = p_psum.tile([128, 256], f32, tag="PP")
                for l in range(l_first, l_last + 1):
                    hsrc = h_prev if l == 0 else h1_prev
                    for q in range(2):
                        c = col(l, q)
                        rs = slice(l * 128 + q * 64, l * 128 + (q + 1) * 64)
                        plast = (l == l_last) and (q == 1)
                        bmm("P", PP[:, c], W["id6"][:, :], hsrc[:, rs])
                        bmm("P", PP[:, c], W["id1"][:, :], ks[0][:, rs])
                        bmm("P", PP[:, c], W["id2"][:, :], ks[1][:, rs])
                        bmm("P", PP[:, c], W["id2"][:, :], ks[2][:, rs])
                        bmm("P", PP[:, c], W["id1"][:, :], ks[3][:, rs], last=plast)
                nc.scalar.activation(P_sb[:, cs], PP[:, cs], AF.Copy)
                P2_sb = s_pool.tile([128, 256], bf16, tag="P2")
                nc.scalar.activation(P2_sb[:, cs], PP[:, cs], AF.Square)

                # -------- LN stats: sums over H via ones-matmul ----------
                stp = st_psum.tile([1, 512], f32, tag="st")
                for l in range(l_first, l_last + 1):
                    slast = (l == l_last)
                    bmm("S", stp[:, l * 64:(l + 1) * 64], W["ones_red"][:, :], P_sb[:, col(l, 0)])
                    bmm("S", stp[:, l * 64:(l + 1) * 64], W["ones_red"][:, :], P_sb[:, col(l, 1)])
                    bmm("S", stp[:, 256 + l * 64:256 + (l + 1) * 64], W["ones_red"][:, :], P2_sb[:, col(l, 0)])
                    bmm("S", stp[:, 256 + l * 64:256 + (l + 1) * 64], W["ones_red"][:, :], P2_sb[:, col(l, 1)], last=slast)
                rlo = 0 if do0 else 64
                rhi = 128 if do1 else 64
                rs_ = slice(rlo, rhi)
                sx = row_pool.tile([1, 128], f32, tag="sx")
                nc.vector.tensor_scalar(out=sx[:, rs_], in0=stp[0:1, rs_], scalar1=1.0, scalar2=None, op0=OP.mult)
                ms = row_pool.tile([1, 128], f32, tag="ms")
                nc.vector.scalar_tensor_tensor(out=ms[:, rs_], in0=sx[:, rs_], scalar=0.0, in1=sx[:, rs_], op0=OP.add, op1=OP.mult)
                t1 = row_pool.tile([1, 128], f32, tag="t1")
                nc.vector.tensor_scalar(out=t1[:, rs_], in0=stp[0:1, 256 + rlo:256 + rhi], scalar1=N_H, scalar2=N_H * N_H * 36.0 * LN_EPS, op0=OP.mult, op1=OP.add)
                vv = row_pool.tile([1, 128], f32, tag="vv")
                nc.vector.scalar_tensor_tensor(out=vv[:, rs_], in0=ms[:, rs_], scalar=-1.0, in1=t1[:, rs_], op0=OP.mult, op1=OP.add)
                w_ = row_pool.tile([1, 128], f32, tag="w")
                nc.vector.tensor_scalar(out=w_[:, rs_], in0=vv[:, rs_], scalar1=SEED_B, scalar2=SEED_A, op0=OP.mult, op1=OP.add)
                y = row_pool.tile([1, 128], f32, tag="y")
                yb = row_pool.tile([1, 128], bf16, tag="yb")
                nc.vector.reciprocal_approx_fast(out=y[:, rs_], in_=w_[:, rs_])
                yy = row_pool.tile([1, 128], f32, tag="yy")
                tn = row_pool.tile([1, 128], f32, tag="tn")
                for _ in range(RSQRT_ITERS):
                    nc.vector.scalar_tensor_tensor(out=yy[:, rs_], in0=y[:, rs_], scalar=0.0, in1=y[:, rs_], op0=OP.add, op1=OP.mult)
                    nc.vector.scalar_tensor_tensor(out=tn[:, rs_], in0=yy[:, rs_], scalar=-0.5, in1=vv[:, rs_], op0=OP.mult, op1=OP.mult)
                    nc.vector.scalar_tensor_tensor(out=y[:, rs_], in0=tn[:, rs_], scalar=1.5, in1=y[:, rs_], op0=OP.add, op1=OP.mult)
                nc.vector.tensor_scalar(out=yb[:, rs_], in0=y[:, rs_], scalar1=0.0, scalar2=None, op0=OP.add)
                # nm = sum(x)*y -> row1 of trhs (t' = lnb + (-lng)*mu*rstd)
                nc.vector.scalar_tensor_tensor(out=trhs[0:1, rs_], in0=sx[:, rs_], scalar=0.0, in1=y[:, rs_], op0=OP.add, op1=OP.mult)

                # -------- broadcasts: s' = lngN (x) y ; t' = lnb(x)1 + (-lng)(x)nm
                bcp = bc_psum.tile([128, 512], f32, tag="bc")
                for l in range(l_first, l_last + 1):
                    for q in range(2):
                        lq = (2 * l + q) * 128
                        blast = (l == l_last) and (q == 1)
                        bmm("B", bcp[:, col(l, q)], W["lngN"][:, lq:lq + 128], yb[0:1, l * 64:(l + 1) * 64])
                        bmm("B", bcp[:, 256 + l * 128 + q * 64:256 + l * 128 + (q + 1) * 64], W["lnT"][:, lq:lq + 128], trhs[:, l * 64:(l + 1) * 64], last=blast)
                z = s_pool.tile([128, 256], f32, tag="z")
                nc.vector.scalar_tensor_tensor(out=z[:, cs], in0=P_sb[:, cs], scalar=0.0, in1=bcp[:, lo:hi], op0=OP.add, op1=OP.mult)
                z2 = s_pool.tile([128, 256], f32, tag="z2")
                nc.vector.scalar_tensor_tensor(out=z2[:, cs], in0=z[:, cs], scalar=0.0, in1=bcp[:, 256 + lo:256 + hi], op0=OP.add, op1=OP.add)
                nc.scalar.activation(h_next[:, cs], z2[:, cs], AF.Tanh)

            # ----- tick 0: layer0 only, zero states; zero l1 half of slot0
            xc0 = xc_pool.tile([97, BC], bf16, tag="xc")
            nc.sync.dma_start(xc0[:], xcat[0])
            nc.vector.memset(ring[0][:, 128:256], 0.0)
            emit_tick(xc0, hz, hz, ring[0], True, False)

            # ----- main loop: ticks 1 .. loop_hi-1 ------------------------
            with tc.For_i(1, loop_hi, unroll) as iv:
                xslab = xcat[ds(iv, unroll)]
                for u in range(unroll):
                    tau = u + 1  # slot parity: tick tau=1+8j+u -> reads slot((u)%8) writes slot((u+1)%8)
                    xcu = xc_pool.tile([97, BC], bf16, tag="xc")
                    nc.sync.dma_start(xcu[:], xslab[u])
                    emit_tick(xcu, ring[u % 8], ring[u % 8], ring[(u + 1) % 8], True, True)

            # ----- tail ticks: loop_hi .. T_run ---------------------------
            for tau in range(loop_hi, T_run + 1):
                do0 = tau < T_run
                if do0:
                    xct = xc_pool.tile([97, BC], bf16, tag="xc")
                    nc.sync.dma_start(xct[:], xcat[tau])
                else:
                    xct = None
                emit_tick(xct, slot(tau - 1), slot(tau - 1), slot(tau), do0, True)

            nc.sync.dma_start(hout[:], slot(T_run)[:, 128:256])

    nc.compile()
    return nc


# ---------------------------------------------------------------------------
# Host-side weight prep
# ---------------------------------------------------------------------------

def _prep_weights(inputs):
    import ml_dtypes
    bf = ml_dtypes.bfloat16
    W = {k: np.asarray(v, np.float32) for k, v in inputs.items()}
    out = {}

    def pack_kt(wT):  # [256, 256] -> [128, 512] (kt, m)
        return np.concatenate([wT[0:128, :], wT[128:256, :]], axis=1)

    for l in range(L):
        fin = FS + FC if l == 0 else H
        Wg, Win, Wrec = W[f'Wg{l}'], W[f'Win{l}'], W[f'Wrec{l}']
        bg, tau = W[f'bg{l}'], W[f'tau{l}']
        itau = (1.0 / (softplus_np(tau) + 1.0)).astype(np.float32)
        WgxT = Wg[:, :fin].T            # [fin, 256]
        WghT = Wg[:, fin:].T            # [256, 256]
        WrecT = Wrec.T
        WinT = Win.T                    # [fin, 256]
        if l == 0:
            out["g_x0"] = np.concatenate([WgxT, bg[None, :]], 0).astype(bf)   # [97, 256]
            out["in_x0"] = WinT.astype(bf)                                     # [96, 256]
            out["g_h0"] = pack_kt(WghT).astype(bf)
            out["g_h0h"] = pack_kt(WghT * 0.5).astype(bf)
            out["rec0"] = pack_kt(WrecT).astype(bf)
            out["rec0h"] = pack_kt(WrecT * 0.5).astype(bf)
            dn = np.zeros((128, 256), np.float32)
            dnh = np.zeros((128, 256), np.float32)
            for q in range(2):
                dn[:, q * 128:(q + 1) * 128] = np.diag(-itau[q * 128:(q + 1) * 128])
                dnh[:, q * 128:(q + 1) * 128] = np.diag(-0.5 * itau[q * 128:(q + 1) * 128])
            out["dneg0"] = dn.astype(bf)
            out["dneg0h"] = dnh.astype(bf)
        else:
            out["g_x1"] = pack_kt(WgxT).astype(bf)
            out["bg1row"] = bg[None, :].astype(bf)
            out["in_x1"] = pack_kt(WinT).astype(bf)
            out["g_h1"] = pack_kt(WghT).astype(bf)
            out["g_h1h"] = pack_kt(WghT * 0.5).astype(bf)
            out["rec1"] = pack_kt(WrecT).astype(bf)
            out["rec1h"] = pack_kt(WrecT * 0.5).astype(bf)
            dn = np.zeros((128, 256), np.float32)
            dnh = np.zeros((128, 256), np.float32)
            for q in range(2):
                dn[:, q * 128:(q + 1) * 128] = np.diag(-itau[q * 128:(q + 1) * 128])
                dnh[:, q * 128:(q + 1) * 128] = np.diag(-0.5 * itau[q * 128:(q + 1) * 128])
            out["dneg1"] = dn.astype(bf)
            out["dneg1h"] = dnh.astype(bf)
    out["id6"] = (6.0 * np.eye(128, dtype=np.float32)).astype(bf)
    out["id1"] = np.eye(128, dtype=np.float32).astype(bf)
    out["id2"] = (2.0 * np.eye(128, dtype=np.float32)).astype(bf)
    lnT = np.zeros((2, 512), np.float32)
    lngN = np.zeros((1, 512), np.float32)
    for l in range(L):
        lng, lnb = W[f'lng{l}'], W[f'lnb{l}']
        for q in range(2):
            lq = (2 * l + q) * 128
            lnT[0, lq:lq + 128] = -lng[q * 128:(q + 1) * 128]
            lnT[1, lq:lq + 128] = lnb[q * 128:(q + 1) * 128]
            lngN[0, lq:lq + 128] = lng[q * 128:(q + 1) * 128] * N_H
    out["lnT"] = lnT.astype(bf)
    out["lngN"] = lngN.astype(bf)
    out["ones_red"] = np.ones((128, 1), np.float32).astype(bf)
    out["ones2"] = np.ones((2, 128), np.float32).astype(bf)
    return out


def _prep_core_inputs(inputs, wpack, core, T_run=T):
    seq = np.asarray(inputs['seq_features'], np.float32)   # [B,T,FS]
    ctx = np.asarray(inputs['context_features'], np.float32)  # [B,FC]
    bsl = slice(core * BC, (core + 1) * BC)
    import ml_dtypes
    xc = np.empty((T_run, 97, BC), np.float32)
    xc[:, 0:64, :] = seq[bsl, :T_run].transpose(1, 2, 0)     # [T, FS, BC]
    xc[:, 64:96, :] = ctx[bsl].T[None, :, :]
    xc[:, 96, :] = 1.0
    m = {"xcat": xc.astype(ml_dtypes.bfloat16)}
    m.update(wpack)
    return m


def _head(inputs, h1):  # h1: [B, H] final layer1 state
    cW1 = np.asarray(inputs['cW1'], np.float32)
    cb1 = np.asarray(inputs['cb1'], np.float32)
    cW2 = np.asarray(inputs['cW2'], np.float32)
    cb2 = np.asarray(inputs['cb2'], np.float32)
    hid = np.maximum(h1 @ cW1.T + cb1, 0)
    return (hid @ cW2.T + cb2).squeeze(-1)


_CACHE = {}


def kernel(**inputs):
    if "nc" not in _CACHE:
        _CACHE["nc"] = build_module(T, UNROLL)
    nc = _CACHE["nc"]
    from concourse.bass_utils import run_bass_kernel_spmd
    wpack = _prep_weights(inputs)
    in_maps = [_prep_core_inputs(inputs, wpack, c) for c in range(NCORES)]
    do_trace = os.environ.get("BASS_KERNEL_TRACE") == "1"
    r = run_bass_kernel_spmd(nc, in_maps, list(range(NCORES)), trace=do_trace)
    res = r.results
    if do_trace:
        _CACHE["exec_ns"] = r.exec_time_ns
        if r.instructions_and_trace is not None:
            _CACHE["trace_path"] = r.instructions_and_trace[1]
    h1 = np.empty((B, H), np.float32)
    for c in range(NCORES):
        ht = np.asarray(res[c]["hout"], np.float32)  # [128, (q,b)]
        bsl = slice(c * BC, (c + 1) * BC)
        for q in range(2):
            h1[bsl, q * 128:(q + 1) * 128] = ht[:, q * 64:(q + 1) * 64].T
    return _head(inputs, h1).astype(np.float32)


if __name__ == "__main__":
    pass



# revision 10
# speedup vs baseline: 1.1724x; 1.1724x over previous
"""Trainium2 Bass kernel for nn_LiquidNeuralNetwork (v2).

Strategy: data-parallel over batch (8 cores x 64). Per core, a fully on-chip
recurrence over T=512 steps with the two LTC layers run as TWO INDEPENDENT
INSTRUCTION STREAMS (layer0 at tick t, layer1 at tick t-1), interleaved at
RK4-stage granularity so the tensor-engine matmuls of one stream hide the
scalar/vector latency of the other.

Per-layer tiles are [128 part = h%128, free = m*64 + b] (m = h//128 output
half, b = batch-in-core). The gate sigmoid(tanh(u)) is replaced by the fitted
a*tanh(b*u)+0.5 (max abs err 6.7e-4) so each RK4 stage costs one ScalarE tanh
plus one fused DVE (t*a+0.5)*R multiply. The LayerNorm rsqrt runs as 4 custom
DVE row ops (linear-seed + reciprocal_approx_fast + 2 Newton steps in w-form).
RK4 P-assembly and dr-deltas run on the otherwise-idle GpSimd engine.
"""

import os
import sys
import numpy as np

sys.path.insert(0, "/opt/trn_rl_repo")

B, T, FS, FC, H, L = 512, 512, 64, 32, 256, 2
LN_EPS = 1e-5
NCORES = 8
BC = B // NCORES       # 64 batch per core
UNROLL = 8
N_H = 256.0

# gate fit: sigmoid(tanh(u)) ~= GATE_A * tanh(GATE_B * u) + 0.5
GATE_A, GATE_B = 0.230386, 1.072557

# rsqrt(vv) over observed vv range [1.0e5, 1.7e6] with 4x safety margin
VLO, VHI = 2.5e4, 6.8e6
_VC = float(np.sqrt(VLO * VHI))
SEED_B = 0.5 / float(np.sqrt(_VC))
SEED_A = 1.0 / (4.0 * SEED_B)
# w = SEED_B*(N*S2 + 36*N^2*eps - S1^2) + SEED_A ~= sqrt(vv)
VVW_C0 = SEED_B * N_H
VVW_C1 = SEED_A + SEED_B * 36.0 * N_H * N_H * LN_EPS
VVW_C2 = SEED_B
# Newton in w-form: y' = (1.5 - w*y^2*(0.5/b) + (0.5a/b)*y^2) * y
NRW_C0 = 1.5
NRW_C1 = 0.5 / SEED_B
NRW_C2 = 0.5 * SEED_A / SEED_B

USE_GPSIMD = os.environ.get("LNN_NO_GPSIMD") != "1"


def softplus_np(x):
    return np.log1p(np.exp(-np.abs(x))) + np.maximum(x, 0)


# ---------------------------------------------------------------------------
# Custom DVE ops
# ---------------------------------------------------------------------------

_OPS_CACHE = {}


def _get_custom_ops():
    if _OPS_CACHE:
        return _OPS_CACHE
    from concourse.dve_spec import Spec, Src0, Src1, C0, C1, C2, lower, sq
    from concourse.dve_spec import _has_src1
    from concourse.dve_uop import DveOpSpec
    from concourse import dve_ops

    _m = sq(Src1)
    defs = {
        # out = (in0*s0 + s1) * in1   -- gate affine folded into the R-multiply
        "GATE_MUL_LNN": (
            (Src0 * C0 + C1) * Src1,
            lambda in0, in1, s0, s1, imm2: (
                (in0.astype(np.float32) * s0 + s1) * in1
            ).astype(np.float32),
        ),
        # out = (in0*s0 + s1) - in1^2 * imm2   -- w = b*(N*S2 + c - S1^2) + a
        "VV_W_LNN": (
            (Src0 * C0 + C1) - sq(Src1) * C2,
            lambda in0, in1, s0, s1, imm2: (
                (in0.astype(np.float32) * s0 + s1)
                - np.square(in1.astype(np.float32)) * imm2
            ).astype(np.float32),
        ),
        # out = (s0 - in0*in1^2*s1 + imm2*in1^2) * in1  -- Newton step, w-form
        "NR_W_LNN": (
            (C0 - Src0 * _m * C1 + C2 * _m) * Src1,
            lambda in0, in1, s0, s1, imm2: (
                (
                    s0
                    - in0.astype(np.float32) * np.square(in1.astype(np.float32)) * s1
                    + imm2 * np.square(in1.astype(np.float32))
                )
                * in1
            ).astype(np.float32),
        ),
    }
    for name, (body, ref) in defs.items():
        if name in dve_ops._SUB_OPCODE_FOR_NAME:
            _OPS_CACHE[name] = next(o for o in dve_ops.OPS if o.name == name)
            continue
        spec = Spec(body=body, reference=ref)
        opcode = dve_ops._CUSTOM_DVE_ROW_BASE + len(dve_ops.OPS)
        shas = {}
        for ver in ("v3", "v4"):
            shas[ver] = DveOpSpec(
                name=name,
                opcode=opcode,
                uops=lower(spec, ver=ver),
                rd1_en=_has_src1(spec),
            ).sha(ver)
        op = dve_ops.DveOp(name, spec, subdim=False, uops_sha=shas)
        dve_ops.OPS.append(op)
        dve_ops._SUB_OPCODE_FOR_NAME[name] = opcode
        dve_ops.CUSTOM_DVE_SPECS[name] = spec
        _OPS_CACHE[name] = op
    return _OPS_CACHE


# ---------------------------------------------------------------------------
# Bass module builder
# ---------------------------------------------------------------------------

def build_module(T_run=T, unroll=UNROLL):
    import concourse.bass as bass
    import concourse.mybir as mybir
    from concourse import tile, bacc
    from concourse.bass import ds

    ops = _get_custom_ops()
    GATE_MUL = ops["GATE_MUL_LNN"]
    VV_W = ops["VV_W_LNN"]
    NR_W = ops["NR_W_LNN"]

    f32 = mybir.dt.float32
    bf16 = mybir.dt.bfloat16
    AF = mybir.ActivationFunctionType
    OP = mybir.AluOpType

    nc = bacc.Bacc(None, target_bir_lowering=False)

    xcat = nc.declare_dram_parameter("xcat", [T_run, 97, BC], bf16, isOutput=False)
    wdecl = {}

    def wparam(name, shape):
        wdecl[name] = nc.declare_dram_parameter(name, shape, bf16, isOutput=False)
        return wdecl[name]

    wparam("g_x0", [97, 256])
    wparam("in_x0", [96, 256])
    wparam("g_h0", [128, 512])
    wparam("g_h0h", [128, 512])
    wparam("rec0", [128, 512])
    wparam("rec0h", [128, 512])
    wparam("dneg0", [128, 256])
    wparam("dneg0h", [128, 256])
    wparam("g_x1", [128, 512])
    wparam("bg1row", [1, 256])
    wparam("in_x1", [128, 512])
    wparam("g_h1", [128, 512])
    wparam("g_h1h", [128, 512])
    wparam("rec1", [128, 512])
    wparam("rec1h", [128, 512])
    wparam("dneg1", [128, 256])
    wparam("dneg1h", [128, 256])
    wparam("lnT", [2, 512])      # rows: [-lng ; lnb], col blocks (2l+m)*128
    wparam("lngN", [1, 512])     # lng * N
    wparam("ones_red", [128, 1])
    wparam("ones2", [2, 64])
    hout = nc.declare_dram_parameter("hout", [128, 128], bf16, isOutput=True)

    # slots 1..504 in the hw loop; 0 head; 505..512 tail
    LOOP_LO, LOOP_HI = 1, 505
    assert (LOOP_HI - LOOP_LO) % unroll == 0

    with tile.TileContext(nc) as tc:
        from contextlib import ExitStack
        with ExitStack() as ctx:
            singles = ctx.enter_context(tc.tile_pool(name="singles", bufs=1))
            xc_pool = ctx.enter_context(tc.tile_pool(name="xc", bufs=12))
            pools = {}
            for l in range(L):
                for nm, bufs in [("tg", 2), ("mm", 2), ("k", 5), ("dr", 2),
                                 ("e", 5), ("P", 2), ("P2", 2), ("z", 2),
                                 ("z2", 2), ("row", 6)]:
                    pools[(nm, l)] = ctx.enter_context(
                        tc.tile_pool(name=f"{nm}{l}", bufs=bufs))
            # PSUM pools are bank-granular (2KB/partition per buf): one bank
            # each for G/R/V per layer, bc+stats share the tail bank.
            gps = [ctx.enter_context(tc.tile_pool(name=f"g{l}ps", bufs=1, space="PSUM")) for l in range(L)]
            rps = [ctx.enter_context(tc.tile_pool(name=f"r{l}ps", bufs=1, space="PSUM")) for l in range(L)]
            vps = [ctx.enter_context(tc.tile_pool(name=f"v{l}ps", bufs=1, space="PSUM")) for l in range(L)]
            tailps = [ctx.enter_context(tc.tile_pool(name=f"tail{l}ps", bufs=1, space="PSUM")) for l in range(L)]

            # ---- resident weights ---------------------------------------
            W = {}
            for name, dram in wdecl.items():
                t_ = singles.tile(list(dram.shape), bf16, name=name, tag=name)
                nc.sync.dma_start(t_[:], dram[:])
                W[name] = t_

            rings = [[singles.tile([128, 128], bf16, name=f"ring{l}_{i}", tag=f"ring{l}_{i}")
                      for i in range(4)] for l in range(L)]
            hz = singles.tile([128, 128], bf16)
            nc.vector.memset(hz[:], 0.0)
            nc.vector.memset(rings[1][3][:], 0.0)   # h1[-1] = 0
            onesr = singles.tile([1, BC], bf16)
            nc.vector.memset(onesr[:], 1.0)
            trhs = [singles.tile([2, BC], bf16, name=f"trhs{l}", tag=f"trhs{l}") for l in range(L)]
            for l in range(L):
                nc.sync.dma_start(trhs[l][:, :], wdecl["ones2"][:, :])

            def wt(name, kt, m):
                return W[name][:, kt * 256 + m * 128: kt * 256 + (m + 1) * 128]

            def dneg(l, half, m):
                nm = f"dneg{l}" + ("h" if half else "")
                return W[nm][:, m * 128:(m + 1) * 128]

            # ---- per-slot state carried between emit calls ----------------
            class Part:
                pass

            def stage_mms(st, l, s, xc, x1, h_self, dr):
                """Emit stage-s matmuls for layer l. G first, then tanh can
                issue while R/V stream. Returns the tg tile."""
                def Gc(m):
                    return st.G[:, m * 64:(m + 1) * 64]

                def Rc(m):
                    return st.R[:, m * 64:(m + 1) * 64]

                def Vc(m):
                    return st.V[:, m * 64:(m + 1) * 64]

                def bmm(bank, out_ap, lhsT, rhs, last=False):
                    first = bank not in st.first
                    st.first[bank] = True
                    nc.tensor.matmul(out_ap, lhsT, rhs, start=first, stop=last,
                                     skip_group_check=True)

                sfx = "h" if s in (2, 3) else ""
                if s == 1:
                    if l == 0:
                        for m in range(2):
                            bmm("G", Gc(m), W["g_x0"][:, m * 128:(m + 1) * 128], xc[:, :])
                            for kt in range(2):
                                bmm("G", Gc(m), wt("g_h0", kt, m), h_self[:, kt * 64:(kt + 1) * 64])
                    else:
                        for m in range(2):
                            for kt in range(2):
                                bmm("G", Gc(m), wt("g_x1", kt, m), x1[:, kt * 64:(kt + 1) * 64])
                            bmm("G", Gc(m), W["bg1row"][:, m * 128:(m + 1) * 128], onesr[:, :])
                            for kt in range(2):
                                bmm("G", Gc(m), wt("g_h1", kt, m), h_self[:, kt * 64:(kt + 1) * 64])
                    tg = pools[("tg", l)].tile([128, 128], f32, tag="tg")
                    nc.scalar.activation(tg[:, :], st.G[:, :], AF.Tanh, scale=GATE_B)
                    for m in range(2):
                        for kt in range(2):
                            bmm("R", Rc(m), wt(f"rec{l}", kt, m), h_self[:, kt * 64:(kt + 1) * 64])
                    if l == 0:
                        for m in range(2):
                            bmm("V", Vc(m), W["in_x0"][:, m * 128:(m + 1) * 128], xc[0:96, :])
                            bmm("V", Vc(m), dneg(0, False, m), h_self[:, m * 64:(m + 1) * 64])
                    else:
                        for m in range(2):
                            for kt in range(2):
                                bmm("V", Vc(m), wt("in_x1", kt, m), x1[:, kt * 64:(kt + 1) * 64])
                            bmm("V", Vc(m), dneg(1, False, m), h_self[:, m * 64:(m + 1) * 64])
                else:
                    half = s in (2, 3)
                    last = s == 4
                    for m in range(2):
                        for kt in range(2):
                            bmm("G", Gc(m), wt(f"g_h{l}{sfx}", kt, m),
                                dr[:, kt * 64:(kt + 1) * 64], last=last and m == 1 and kt == 1)
                    tg = pools[("tg", l)].tile([128, 128], f32, tag="tg")
                    nc.scalar.activation(tg[:, :], st.G[:, :], AF.Tanh, scale=GATE_B)
                    for m in range(2):
                        for kt in range(2):
                            bmm("R", Rc(m), wt(f"rec{l}{sfx}", kt, m),
                                dr[:, kt * 64:(kt + 1) * 64], last=last and m == 1 and kt == 1)
                    for m in range(2):
                        bmm("V", Vc(m), dneg(l, half, m),
                            dr[:, m * 64:(m + 1) * 64], last=last and m == 1)
                return tg

            eng2 = nc.gpsimd if USE_GPSIMD else nc.vector

            def g_sub(out, a, b):  # out = a - b  (SBUF-only operands)
                if USE_GPSIMD:
                    eng2.tensor_sub(out, a, b)
                else:
                    nc.vector.scalar_tensor_tensor(out=out, in0=b, scalar=-1.0,
                                                   in1=a, op0=OP.mult, op1=OP.add)

            def g_add(out, a, b):
                if USE_GPSIMD:
                    eng2.tensor_add(out, a, b)
                else:
                    nc.vector.scalar_tensor_tensor(out=out, in0=a, scalar=0.0,
                                                   in1=b, op0=OP.add, op1=OP.add)

            def g_smul(out, a, s):
                eng2.tensor_scalar_mul(out, a, s)

            def g_mul(out, a, b):
                if USE_GPSIMD:
                    eng2.tensor_mul(out, a, b)
                else:
                    nc.vector.scalar_tensor_tensor(out=out, in0=a, scalar=0.0,
                                                   in1=b, op0=OP.add, op1=OP.mult)

            def emit_stage(st, l, s, xc=None, x1=None, h_self=None):
                dr = st.dr_next
                tg = stage_mms(st, l, s, xc, x1, h_self, dr)
                mm = pools[("mm", l)].tile([128, 128], f32, tag="mm")
                nc.vector._custom_dve(GATE_MUL, out=mm[:, :], in0=tg[:, :],
                                      in1=st.R[:, :], s0=GATE_A, s1=0.5)
                kk = pools[("k", l)].tile([128, 128], bf16, tag=f"k{s}")
                nc.vector.scalar_tensor_tensor(out=kk[:, :], in0=mm[:, :], scalar=0.0,
                                               in1=st.V[:, :], op0=OP.add, op1=OP.add)
                st.ks.append(kk)
                if s == 1:
                    st.dr_next = kk
                elif s == 2:
                    dr2 = pools[("dr", l)].tile([128, 128], bf16, tag="dr")
                    g_sub(dr2[:, :], kk[:, :], st.ks[0][:, :])
                    st.dr_next = dr2
                elif s == 3:
                    kh = pools[("e", l)].tile([128, 128], f32, tag="e")
                    g_smul(kh[:, :], st.ks[1][:, :], -0.5)
                    dr3 = pools[("dr", l)].tile([128, 128], bf16, tag="dr")
                    g_add(dr3[:, :], kh[:, :], kk[:, :])
                    st.dr_next = dr3
                    e1 = pools[("e", l)].tile([128, 128], f32, tag="e")
                    g_add(e1[:, :], st.ks[1][:, :], kk[:, :])
                    st.e1 = e1

            def emit_tail_a(st, l, h_self):
                """k4 already in st.ks[3]; assemble P = 6h + k1+2k2+2k3+k4,
                square it, and emit the LN-stat matmuls."""
                e2 = pools[("e", l)].tile([128, 128], f32, tag="e")
                g_smul(e2[:, :], st.e1[:, :], 2.0)
                e3 = pools[("e", l)].tile([128, 128], f32, tag="e")
                g_add(e3[:, :], e2[:, :], st.ks[0][:, :])
                e4 = pools[("e", l)].tile([128, 128], f32, tag="e")
                nc.vector.scalar_tensor_tensor(out=e4[:, :], in0=h_self[:, :], scalar=6.0,
                                               in1=e3[:, :], op0=OP.mult, op1=OP.add)
                P = pools[("P", l)].tile([128, 128], bf16, tag="P")
                nc.vector.scalar_tensor_tensor(out=P[:, :], in0=st.ks[3][:, :], scalar=0.0,
                                               in1=e4[:, :], op0=OP.add, op1=OP.add)
                P2 = pools[("P2", l)].tile([128, 128], bf16, tag="P2")
                nc.scalar.activation(P2[:, :], P[:, :], AF.Square)
                tl = st.tail
                nc.tensor.matmul(tl[0:1, 256:320], W["ones_red"][:, :], P[:, 0:64],
                                 start=True, stop=False, skip_group_check=True)
                nc.tensor.matmul(tl[0:1, 256:320], W["ones_red"][:, :], P[:, 64:128],
                                 start=False, stop=True, skip_group_check=True)
                nc.tensor.matmul(tl[0:1, 320:384], W["ones_red"][:, :], P2[:, 0:64],
                                 start=True, stop=False, skip_group_check=True)
                nc.tensor.matmul(tl[0:1, 320:384], W["ones_red"][:, :], P2[:, 64:128],
                                 start=False, stop=True, skip_group_check=True)
                st.P = P

            def emit_tail_b(st, l, h_next):
                tl = st.tail
                s1_ap = tl[0:1, 256:320]
                s2_ap = tl[0:1, 320:384]
                rp = pools[("row", l)]
                sx = rp.tile([1, BC], f32, tag="sx")
                nc.scalar.activation(sx[:, :], s1_ap, AF.Copy)
                w = rp.tile([1, BC], f32, tag="w")
                nc.vector._custom_dve(VV_W, out=w[:, :], in0=s2_ap,
                                      in1=sx[:, :], s0=VVW_C0, s1=VVW_C1, imm2=VVW_C2)
                y0 = rp.tile([1, BC], f32, tag="y0")
                nc.vector.reciprocal_approx_fast(out=y0[:, :], in_=w[:, :])
                y1 = rp.tile([1, BC], f32, tag="y1")
                nc.vector._custom_dve(NR_W, out=y1[:, :], in0=w[:, :], in1=y0[:, :],
                                      s0=NRW_C0, s1=NRW_C1, imm2=NRW_C2)
                y2 = rp.tile([1, BC], bf16, tag="y2")
                nc.vector._custom_dve(NR_W, out=y2[:, :], in0=w[:, :], in1=y1[:, :],
                                      s0=NRW_C0, s1=NRW_C1, imm2=NRW_C2)
                g_mul(trhs[l][0:1, :], sx[:, :], y2[:, :])
                for m in range(2):
                    lq = (2 * l + m) * 128
                    nc.tensor.matmul(tl[:, m * 64:(m + 1) * 64], W["lngN"][:, lq:lq + 128],
                                     y2[0:1, :], start=True, stop=True, skip_group_check=True)
                    nc.tensor.matmul(tl[:, 128 + m * 64:128 + (m + 1) * 64], W["lnT"][:, lq:lq + 128],
                                     trhs[l][:, :], start=True, stop=True, skip_group_check=True)
                z = pools[("z", l)].tile([128, 128], f32, tag="z")
                nc.vector.scalar_tensor_tensor(out=z[:, :], in0=st.P[:, :], scalar=0.0,
                                               in1=tl[:, 0:128], op0=OP.add, op1=OP.mult)
                z2 = pools[("z2", l)].tile([128, 128], f32, tag="z2")
                nc.vector.scalar_tensor_tensor(out=z2[:, :], in0=z[:, :], scalar=0.0,
                                               in1=tl[:, 128:256], op0=OP.add, op1=OP.add)
                nc.scalar.activation(h_next[:, :], z2[:, :], AF.Tanh)

            def new_part(l):
                st = Part()
                st.G = gps[l].tile([128, 128], f32, tag="G")
                st.R = rps[l].tile([128, 128], f32, tag="R")
                st.V = vps[l].tile([128, 128], f32, tag="V")
                st.tail = tailps[l].tile([128, 512], f32, tag="tail")
                st.first = {}
                st.ks = []
                st.dr_next = None
                return st

            def emit_slot(tau_mod4, xc, do_a, do_b, a_self, b_self, b_x1,
                          a_next, b_next):
                """Emit one interleaved slot. a_self = h0[tau-1] tile,
                b_self = h1[tau-2], b_x1 = h0[tau-1] (layer1 input),
                a_next/b_next = output ring tiles."""
                sa = new_part(0) if do_a else None
                sb = new_part(1) if do_b else None
                for s in range(1, 5):
                    if do_a:
                        emit_stage(sa, 0, s, xc=xc, h_self=a_self)
                    if do_b:
                        emit_stage(sb, 1, s, x1=b_x1, h_self=b_self)
                if do_a:
                    emit_tail_a(sa, 0, a_self)
                if do_b:
                    emit_tail_a(sb, 1, b_self)
                if do_a:
                    emit_tail_b(sa, 0, a_next)
                if do_b:
                    emit_tail_b(sb, 1, b_next)

            # ---- head: slot 0 (A only, zero state) ------------------------
            xc0 = xc_pool.tile([97, BC], bf16, tag="xc")
            nc.sync.dma_start(xc0[:], xcat[0])
            emit_slot(0, xc0, True, False, hz, None, None, rings[0][0], None)

            # ---- main loop: slots 1..504 ---------------------------------
            with tc.For_i(LOOP_LO, LOOP_HI, unroll) as iv:
                xslab = xcat[ds(iv, unroll)]
                xcu = []
                for u in range(unroll):
                    t_ = xc_pool.tile([97, BC], bf16, tag="xc")
                    nc.sync.dma_start(t_[:], xslab[u])
                    xcu.append(t_)
                for u in range(unroll):
                    tau = 1 + u  # actual slot = iv+u; (iv+u) % 4 == (1+u) % 4
                    r = rings[0]
                    r1 = rings[1]
                    emit_slot(tau % 4, xcu[u], True, True,
                              r[(tau - 1) % 4], r1[(tau - 2) % 4], r[(tau - 1) % 4],
                              r[tau % 4], r1[(tau - 1) % 4])

            # ---- tail: slots 505..512 ------------------------------------
            for tau in range(LOOP_HI, T_run + 1):
                do_a = tau < T_run
                if do_a:
                    xct = xc_pool.tile([97, BC], bf16, tag="xc")
                    nc.sync.dma_start(xct[:], xcat[tau])
                else:
                    xct = None
                r, r1 = rings[0], rings[1]
                emit_slot(tau % 4, xct, do_a, True,
                          r[(tau - 1) % 4], r1[(tau - 2) % 4], r[(tau - 1) % 4],
                          r[tau % 4] if do_a else None, r1[(tau - 1) % 4])

            nc.sync.dma_start(hout[:], rings[1][(T_run - 1) % 4][:])

    nc.compile()
    return nc


# ---------------------------------------------------------------------------
# Host-side weight prep
# ---------------------------------------------------------------------------

def _prep_weights(inputs):
    import ml_dtypes
    bf = ml_dtypes.bfloat16
    W = {k: np.asarray(v, np.float32) for k, v in inputs.items()}
    out = {}

    def pack_kt(wT):  # [256, 256] -> [128, 512] (kt, m)
        return np.concatenate([wT[0:128, :], wT[128:256, :]], axis=1)

    for l in range(L):
        fin = FS + FC if l == 0 else H
        Wg, Win, Wrec = W[f'Wg{l}'], W[f'Win{l}'], W[f'Wrec{l}']
        bg, tau = W[f'bg{l}'], W[f'tau{l}']
        itau = (1.0 / (softplus_np(tau) + 1.0)).astype(np.float32)
        WgxT = Wg[:, :fin].T
        WghT = Wg[:, fin:].T
        WrecT = Wrec.T
        WinT = Win.T
        dn = np.zeros((128, 256), np.float32)
        dnh = np.zeros((128, 256), np.float32)
        for m in range(2):
            dn[:, m * 128:(m + 1) * 128] = np.diag(-itau[m * 128:(m + 1) * 128])
            dnh[:, m * 128:(m + 1) * 128] = np.diag(-0.5 * itau[m * 128:(m + 1) * 128])
        if l == 0:
            out["g_x0"] = np.concatenate([WgxT, bg[None, :]], 0).astype(bf)
            out["in_x0"] = WinT.astype(bf)
            out["g_h0"] = pack_kt(WghT).astype(bf)
            out["g_h0h"] = pack_kt(WghT * 0.5).astype(bf)
            out["rec0"] = pack_kt(WrecT).astype(bf)
            out["rec0h"] = pack_kt(WrecT * 0.5).astype(bf)
            out["dneg0"] = dn.astype(bf)
            out["dneg0h"] = dnh.astype(bf)
        else:
            out["g_x1"] = pack_kt(WgxT).astype(bf)
            out["bg1row"] = bg[None, :].astype(bf)
            out["in_x1"] = pack_kt(WinT).astype(bf)
            out["g_h1"] = pack_kt(WghT).astype(bf)
            out["g_h1h"] = pack_kt(WghT * 0.5).astype(bf)
            out["rec1"] = pack_kt(WrecT).astype(bf)
            out["rec1h"] = pack_kt(WrecT * 0.5).astype(bf)
            out["dneg1"] = dn.astype(bf)
            out["dneg1h"] = dnh.astype(bf)
    lnT = np.zeros((2, 512), np.float32)
    lngN = np.zeros((1, 512), np.float32)
    for l in range(L):
        lng, lnb = W[f'lng{l}'], W[f'lnb{l}']
        for m in range(2):
            lq = (2 * l + m) * 128
            lnT[0, lq:lq + 128] = -lng[m * 128:(m + 1) * 128]
            lnT[1, lq:lq + 128] = lnb[m * 128:(m + 1) * 128]
            lngN[0, lq:lq + 128] = lng[m * 128:(m + 1) * 128] * N_H
    out["lnT"] = lnT.astype(bf)
    out["lngN"] = lngN.astype(bf)
    out["ones_red"] = np.ones((128, 1), np.float32).astype(bf)
    out["ones2"] = np.ones((2, 64), np.float32).astype(bf)
    return out


def _prep_core_inputs(inputs, wpack, core, T_run=T):
    seq = np.asarray(inputs['seq_features'], np.float32)
    ctx = np.asarray(inputs['context_features'], np.float32)
    bsl = slice(core * BC, (core + 1) * BC)
    import ml_dtypes
    xc = np.empty((T_run, 97, BC), np.float32)
    xc[:, 0:64, :] = seq[bsl, :T_run].transpose(1, 2, 0)
    xc[:, 64:96, :] = ctx[bsl].T[None, :, :]
    xc[:, 96, :] = 1.0
    m = {"xcat": xc.astype(ml_dtypes.bfloat16)}
    m.update(wpack)
    return m


def _head(inputs, h1):
    cW1 = np.asarray(inputs['cW1'], np.float32)
    cb1 = np.asarray(inputs['cb1'], np.float32)
    cW2 = np.asarray(inputs['cW2'], np.float32)
    cb2 = np.asarray(inputs['cb2'], np.float32)
    hid = np.maximum(h1 @ cW1.T + cb1, 0)
    return (hid @ cW2.T + cb2).squeeze(-1)


_CACHE = {}


def kernel(**inputs):
    if "nc" not in _CACHE:
        _CACHE["nc"] = build_module(T, UNROLL)
    nc = _CACHE["nc"]
    from concourse.bass_utils import run_bass_kernel_spmd
    wpack = _prep_weights(inputs)
    in_maps = [_prep_core_inputs(inputs, wpack, c) for c in range(NCORES)]
    do_trace = os.environ.get("BASS_KERNEL_TRACE") == "1"
    r = run_bass_kernel_spmd(nc, in_maps, list(range(NCORES)), trace=do_trace)
    res = r.results
    if do_trace:
        _CACHE["exec_ns"] = r.exec_time_ns
        if r.instructions_and_trace is not None:
            _CACHE["trace_path"] = r.instructions_and_trace[1]
    h1 = np.empty((B, H), np.float32)
    for c in range(NCORES):
        ht = np.asarray(res[c]["hout"], np.float32)  # [128, (m,b)]
        bsl = slice(c * BC, (c + 1) * BC)
        for m in range(2):
            h1[bsl, m * 128:(m + 1) * 128] = ht[:, m * 64:(m + 1) * 64].T
    return _head(inputs, h1).astype(np.float32)


if __name__ == "__main__":
    pass


# revision 13
# speedup vs baseline: 1.5225x; 1.2987x over previous
"""Trainium2 Bass kernel for nn_LiquidNeuralNetwork (v2).

Strategy: data-parallel over batch (8 cores x 64). Per core, a fully on-chip
recurrence over T=512 steps with the two LTC layers run as TWO INDEPENDENT
INSTRUCTION STREAMS (layer0 at tick t, layer1 at tick t-1), interleaved at
RK4-stage granularity so the tensor-engine matmuls of one stream hide the
scalar/vector latency of the other.

Per-layer tiles are [128 part = h%128, free = m*64 + b] (m = h//128 output
half, b = batch-in-core). The gate sigmoid(tanh(u)) is replaced by the fitted
a*tanh(b*u)+0.5 (max abs err 6.7e-4) so each RK4 stage costs one ScalarE tanh
plus one fused DVE (t*a+0.5)*R multiply. The LayerNorm rsqrt runs as 4 custom
DVE row ops (linear-seed + reciprocal_approx_fast + 2 Newton steps in w-form).
RK4 P-assembly and dr-deltas run on the otherwise-idle GpSimd engine.
"""

import os
import sys
import numpy as np

sys.path.insert(0, "/opt/trn_rl_repo")

B, T, FS, FC, H, L = 512, 512, 64, 32, 256, 2
LN_EPS = 1e-5
NCORES = 8
BC = B // NCORES       # 64 batch per core
UNROLL = 8
N_H = 256.0

# gate fit: sigmoid(tanh(u)) ~= GATE_A * tanh(GATE_B * u) + 0.5
GATE_A, GATE_B = 0.230386, 1.072557

# rsqrt(vv) over observed vv range [1.0e5, 1.7e6] with 4x safety margin
VLO, VHI = 2.5e4, 6.8e6
_VC = float(np.sqrt(VLO * VHI))
SEED_B = 0.5 / float(np.sqrt(_VC))
SEED_A = 1.0 / (4.0 * SEED_B)
# w = SEED_B*(N*S2 + 36*N^2*eps - S1^2) + SEED_A ~= sqrt(vv)
VVW_C0 = SEED_B * N_H
VVW_C1 = SEED_A + SEED_B * 36.0 * N_H * N_H * LN_EPS
VVW_C2 = SEED_B
# Newton in w-form: y' = (1.5 - w*y^2*(0.5/b) + (0.5a/b)*y^2) * y
NRW_C0 = 1.5
NRW_C1 = 0.5 / SEED_B
NRW_C2 = 0.5 * SEED_A / SEED_B

USE_GPSIMD = os.environ.get("LNN_NO_GPSIMD") != "1"


def softplus_np(x):
    return np.log1p(np.exp(-np.abs(x))) + np.maximum(x, 0)


# ---------------------------------------------------------------------------
# Custom DVE ops
# ---------------------------------------------------------------------------

_OPS_CACHE = {}


def _get_custom_ops():
    if _OPS_CACHE:
        return _OPS_CACHE
    from concourse.dve_spec import Spec, Src0, Src1, C0, C1, C2, lower, sq
    from concourse.dve_spec import _has_src1
    from concourse.dve_uop import DveOpSpec
    from concourse import dve_ops

    _m = sq(Src1)
    defs = {
        # out = (in0*s0 + s1) * in1   -- gate affine folded into the R-multiply
        "GATE_MUL_LNN": (
            (Src0 * C0 + C1) * Src1,
            lambda in0, in1, s0, s1, imm2: (
                (in0.astype(np.float32) * s0 + s1) * in1
            ).astype(np.float32),
        ),
        # out = (in0*s0 + s1) - in1^2 * imm2   -- w = b*(N*S2 + c - S1^2) + a
        "VV_W_LNN": (
            (Src0 * C0 + C1) - sq(Src1) * C2,
            lambda in0, in1, s0, s1, imm2: (
                (in0.astype(np.float32) * s0 + s1)
                - np.square(in1.astype(np.float32)) * imm2
            ).astype(np.float32),
        ),
        # out = (s0 - in0*in1^2*s1 + imm2*in1^2) * in1  -- Newton step, w-form
        "NR_W_LNN": (
            (C0 - Src0 * _m * C1 + C2 * _m) * Src1,
            lambda in0, in1, s0, s1, imm2: (
                (
                    s0
                    - in0.astype(np.float32) * np.square(in1.astype(np.float32)) * s1
                    + imm2 * np.square(in1.astype(np.float32))
                )
                * in1
            ).astype(np.float32),
        ),
    }
    for name, (body, ref) in defs.items():
        if name in dve_ops._SUB_OPCODE_FOR_NAME:
            _OPS_CACHE[name] = next(o for o in dve_ops.OPS if o.name == name)
            continue
        spec = Spec(body=body, reference=ref)
        opcode = dve_ops._CUSTOM_DVE_ROW_BASE + len(dve_ops.OPS)
        shas = {}
        for ver in ("v3", "v4"):
            shas[ver] = DveOpSpec(
                name=name,
                opcode=opcode,
                uops=lower(spec, ver=ver),
                rd1_en=_has_src1(spec),
            ).sha(ver)
        op = dve_ops.DveOp(name, spec, subdim=False, uops_sha=shas)
        dve_ops.OPS.append(op)
        dve_ops._SUB_OPCODE_FOR_NAME[name] = opcode
        dve_ops.CUSTOM_DVE_SPECS[name] = spec
        _OPS_CACHE[name] = op
    return _OPS_CACHE


# ---------------------------------------------------------------------------
# Bass module builder
# ---------------------------------------------------------------------------

def build_module(T_run=T, unroll=UNROLL):
    import concourse.bass as bass
    import concourse.mybir as mybir
    from concourse import tile, bacc
    from concourse.bass import ds

    ops = _get_custom_ops()
    GATE_MUL = ops["GATE_MUL_LNN"]
    VV_W = ops["VV_W_LNN"]
    NR_W = ops["NR_W_LNN"]

    f32 = mybir.dt.float32
    bf16 = mybir.dt.bfloat16
    AF = mybir.ActivationFunctionType
    OP = mybir.AluOpType

    nc = bacc.Bacc(None, target_bir_lowering=False)

    xcat = nc.declare_dram_parameter("xcat", [T_run, 97, BC], bf16, isOutput=False)
    wdecl = {}

    def wparam(name, shape):
        wdecl[name] = nc.declare_dram_parameter(name, shape, bf16, isOutput=False)
        return wdecl[name]

    wparam("g_x0", [97, 256])
    wparam("in_x0", [96, 256])
    wparam("g_h0", [128, 512])
    wparam("g_h0h", [128, 512])
    wparam("rec0", [128, 512])
    wparam("rec0h", [128, 512])
    wparam("dneg0", [128, 256])
    wparam("dneg0h", [128, 256])
    wparam("g_x1", [128, 512])
    wparam("bg1row", [1, 256])
    wparam("in_x1", [128, 512])
    wparam("g_h1", [128, 512])
    wparam("g_h1h", [128, 512])
    wparam("rec1", [128, 512])
    wparam("rec1h", [128, 512])
    wparam("dneg1", [128, 256])
    wparam("dneg1h", [128, 256])
    wparam("lnT", [2, 512])      # rows: [-lng ; lnb], col blocks (2l+m)*128
    wparam("lngN", [1, 512])     # lng * N
    wparam("ones_red", [128, 1])
    wparam("ones2", [2, 64])
    hout = nc.declare_dram_parameter("hout", [128, 128], bf16, isOutput=True)

    # slots 2..505 in the hw loop; 0..1 head; 506..513 tail
    LOOP_LO, LOOP_HI = 2, 506
    assert (LOOP_HI - LOOP_LO) % unroll == 0

    with tile.TileContext(nc) as tc:
        from contextlib import ExitStack
        with ExitStack() as ctx:
            singles = ctx.enter_context(tc.tile_pool(name="singles", bufs=1))
            xc_pool = ctx.enter_context(tc.tile_pool(name="xc", bufs=12))
            # PSUM: one bank each for G/R/V per layer; bc+stats share a bank.
            gps = [ctx.enter_context(tc.tile_pool(name=f"g{l}ps", bufs=1, space="PSUM")) for l in range(L)]
            rps = [ctx.enter_context(tc.tile_pool(name=f"r{l}ps", bufs=1, space="PSUM")) for l in range(L)]
            vps = [ctx.enter_context(tc.tile_pool(name=f"v{l}ps", bufs=1, space="PSUM")) for l in range(L)]
            tailps = [ctx.enter_context(tc.tile_pool(name=f"tail{l}ps", bufs=1, space="PSUM")) for l in range(L)]

            # ---- resident weights ---------------------------------------
            W = {}
            for name, dram in wdecl.items():
                t_ = singles.tile(list(dram.shape), bf16, name=name, tag=name)
                nc.sync.dma_start(t_[:], dram[:])
                W[name] = t_

            rings = [[singles.tile([128, 128], bf16, name=f"ring{l}_{i}", tag=f"ring{l}_{i}")
                      for i in range(4)] for l in range(L)]
            hz = singles.tile([128, 128], bf16)
            nc.vector.memset(hz[:], 0.0)
            nc.vector.memset(rings[1][3][:], 0.0)   # h1[-1] = 0
            onesr = singles.tile([1, BC], bf16)
            nc.vector.memset(onesr[:], 1.0)
            trhs = [singles.tile([2, BC], bf16, name=f"trhs{l}", tag=f"trhs{l}") for l in range(L)]
            for l in range(L):
                nc.sync.dma_start(trhs[l][:, :], wdecl["ones2"][:, :])

            # static per-stream work tiles (fixed names -> the software
            # pipeline can reference them across the hw-loop back edge)
            def mk(l, nm, dt):
                return singles.tile([128, 128], dt, name=f"{nm}{l}", tag=f"{nm}{l}")

            ST = []
            for l in range(L):
                d = dict(
                    tg=mk(l, "tg", f32), mm=mk(l, "mm", f32),
                    k1=mk(l, "k1", bf16), k2=mk(l, "k2", bf16),
                    k3=mk(l, "k3", bf16), k4=mk(l, "k4", bf16),
                    dr2=mk(l, "dr2", bf16), dr3=mk(l, "dr3", bf16),
                    b1=mk(l, "b1", f32), e1=mk(l, "e1", f32),
                    e2=mk(l, "e2", f32), e3=mk(l, "e3", f32),
                    e4=mk(l, "e4", f32),
                    P=mk(l, "P", bf16), P2=mk(l, "P2", bf16),
                    z=mk(l, "z", f32), z2=mk(l, "z2", f32),
                )
                for nm in ("sx", "w", "y0", "y1"):
                    d[nm] = singles.tile([1, BC], f32, name=nm + str(l), tag=nm + str(l))
                d["y2"] = singles.tile([1, BC], bf16, name=f"y2{l}", tag=f"y2{l}")
                d["G"] = gps[l].tile([128, 128], f32, name=f"G{l}", tag="G")
                d["R"] = rps[l].tile([128, 128], f32, name=f"R{l}", tag="R")
                d["V"] = vps[l].tile([128, 128], f32, name=f"V{l}", tag="V")
                d["tail"] = tailps[l].tile([128, 512], f32, name=f"tail{l}", tag="tail")
                ST.append(d)

            def wt(name, kt, m):
                return W[name][:, kt * 256 + m * 128: kt * 256 + (m + 1) * 128]

            def dneg(l, half, m):
                nm = f"dneg{l}" + ("h" if half else "")
                return W[nm][:, m * 128:(m + 1) * 128]

            eng2 = nc.gpsimd if USE_GPSIMD else nc.vector

            def g_sub(out, a, b):  # out = a - b  (SBUF-only operands)
                if USE_GPSIMD:
                    eng2.tensor_sub(out, a, b)
                else:
                    nc.vector.scalar_tensor_tensor(out=out, in0=b, scalar=-1.0,
                                                   in1=a, op0=OP.mult, op1=OP.add)

            def g_add(out, a, b):
                if USE_GPSIMD:
                    eng2.tensor_add(out, a, b)
                else:
                    nc.vector.scalar_tensor_tensor(out=out, in0=a, scalar=0.0,
                                                   in1=b, op0=OP.add, op1=OP.add)

            def g_mul(out, a, b):
                if USE_GPSIMD:
                    eng2.tensor_mul(out, a, b)
                else:
                    nc.vector.scalar_tensor_tensor(out=out, in0=a, scalar=0.0,
                                                   in1=b, op0=OP.add, op1=OP.mult)

            # per-stream bookkeeping (mm-group first flags survive one tick)
            first_flags = [{}, {}]

            def emit_stage(l, s, xc=None, x1=None, h_self=None):
                """RK4 stage s for layer l: G MMs, gate tanh, R/V MMs, fused
                gate multiply, k, and the next stage's dr."""
                d = ST[l]
                first = first_flags[l]

                def Gc(m):
                    return d["G"][:, m * 64:(m + 1) * 64]

                def Rc(m):
                    return d["R"][:, m * 64:(m + 1) * 64]

                def Vc(m):
                    return d["V"][:, m * 64:(m + 1) * 64]

                def bmm(bank, out_ap, lhsT, rhs, last=False):
                    st_ = bank not in first
                    first[bank] = True
                    nc.tensor.matmul(out_ap, lhsT, rhs, start=st_, stop=last,
                                     skip_group_check=True)

                dr = {1: None, 2: d["k1"], 3: d["dr2"], 4: d["dr3"]}[s]
                if s == 1:
                    first.clear()
                    if l == 0:
                        for m in range(2):
                            bmm("G", Gc(m), W["g_x0"][:, m * 128:(m + 1) * 128], xc[:, :])
                            for kt in range(2):
                                bmm("G", Gc(m), wt("g_h0", kt, m), h_self[:, kt * 64:(kt + 1) * 64])
                    else:
                        for m in range(2):
                            for kt in range(2):
                                bmm("G", Gc(m), wt("g_x1", kt, m), x1[:, kt * 64:(kt + 1) * 64])
                            bmm("G", Gc(m), W["bg1row"][:, m * 128:(m + 1) * 128], onesr[:, :])
                            for kt in range(2):
                                bmm("G", Gc(m), wt("g_h1", kt, m), h_self[:, kt * 64:(kt + 1) * 64])
                    nc.scalar.activation(d["tg"][:, :], d["G"][:, :], AF.Tanh, scale=GATE_B)
                    for m in range(2):
                        for kt in range(2):
                            bmm("R", Rc(m), wt(f"rec{l}", kt, m), h_self[:, kt * 64:(kt + 1) * 64])
                    if l == 0:
                        for m in range(2):
                            bmm("V", Vc(m), W["in_x0"][:, m * 128:(m + 1) * 128], xc[0:96, :])
                            bmm("V", Vc(m), dneg(0, False, m), h_self[:, m * 64:(m + 1) * 64])
                    else:
                        for m in range(2):
                            for kt in range(2):
                                bmm("V", Vc(m), wt("in_x1", kt, m), x1[:, kt * 64:(kt + 1) * 64])
                            bmm("V", Vc(m), dneg(1, False, m), h_self[:, m * 64:(m + 1) * 64])
                else:
                    # stages 2..4 all use the half-scaled weight variants
                    # (stage 4's rhs is dr3' = 2*k3 - k2)
                    last = s == 4
                    for m in range(2):
                        for kt in range(2):
                            bmm("G", Gc(m), wt(f"g_h{l}h", kt, m),
                                dr[:, kt * 64:(kt + 1) * 64], last=last and m == 1 and kt == 1)
                    nc.scalar.activation(d["tg"][:, :], d["G"][:, :], AF.Tanh, scale=GATE_B)
                    for m in range(2):
                        for kt in range(2):
                            bmm("R", Rc(m), wt(f"rec{l}h", kt, m),
                                dr[:, kt * 64:(kt + 1) * 64], last=last and m == 1 and kt == 1)
                    for m in range(2):
                        bmm("V", Vc(m), dneg(l, True, m),
                            dr[:, m * 64:(m + 1) * 64], last=last and m == 1)
                nc.vector._custom_dve(GATE_MUL, out=d["mm"][:, :], in0=d["tg"][:, :],
                                      in1=d["R"][:, :], s0=GATE_A, s1=0.5)
                kk = d[f"k{s}"]
                nc.vector.scalar_tensor_tensor(out=kk[:, :], in0=d["mm"][:, :], scalar=0.0,
                                               in1=d["V"][:, :], op0=OP.add, op1=OP.add)
                if s == 2:
                    g_sub(d["dr2"][:, :], kk[:, :], d["k1"][:, :])
                elif s == 3:
                    g_add(d["b1"][:, :], kk[:, :], kk[:, :])          # 2*k3
                    g_sub(d["dr3"][:, :], d["b1"][:, :], d["k2"][:, :])  # 2k3 - k2
                    g_add(d["e1"][:, :], d["k2"][:, :], kk[:, :])     # k2 + k3

            def emit_tail_a(l, h_self):
                d = ST[l]
                g_add(d["e2"][:, :], d["e1"][:, :], d["e1"][:, :])    # 2(k2+k3)
                g_add(d["e3"][:, :], d["e2"][:, :], d["k1"][:, :])
                nc.vector.scalar_tensor_tensor(out=d["e4"][:, :], in0=h_self[:, :], scalar=6.0,
                                               in1=d["e3"][:, :], op0=OP.mult, op1=OP.add)
                nc.vector.scalar_tensor_tensor(out=d["P"][:, :], in0=d["k4"][:, :], scalar=0.0,
                                               in1=d["e4"][:, :], op0=OP.add, op1=OP.add)
                nc.scalar.activation(d["P2"][:, :], d["P"][:, :], AF.Square)
                tl = d["tail"]
                nc.tensor.matmul(tl[0:1, 256:320], W["ones_red"][:, :], d["P"][:, 0:64],
                                 start=True, stop=False, skip_group_check=True)
                nc.tensor.matmul(tl[0:1, 256:320], W["ones_red"][:, :], d["P"][:, 64:128],
                                 start=False, stop=True, skip_group_check=True)
                nc.tensor.matmul(tl[0:1, 320:384], W["ones_red"][:, :], d["P2"][:, 0:64],
                                 start=True, stop=False, skip_group_check=True)
                nc.tensor.matmul(tl[0:1, 320:384], W["ones_red"][:, :], d["P2"][:, 64:128],
                                 start=False, stop=True, skip_group_check=True)

            def emit_tail_b(l, h_next):
                d = ST[l]
                tl = d["tail"]
                s1_ap = tl[0:1, 256:320]
                s2_ap = tl[0:1, 320:384]
                nc.scalar.activation(d["sx"][:, :], s1_ap, AF.Copy)
                nc.vector._custom_dve(VV_W, out=d["w"][:, :], in0=s2_ap,
                                      in1=d["sx"][:, :], s0=VVW_C0, s1=VVW_C1, imm2=VVW_C2)
                nc.vector.reciprocal_approx_fast(out=d["y0"][:, :], in_=d["w"][:, :])
                nc.vector._custom_dve(NR_W, out=d["y1"][:, :], in0=d["w"][:, :], in1=d["y0"][:, :],
                                      s0=NRW_C0, s1=NRW_C1, imm2=NRW_C2)
                nc.vector._custom_dve(NR_W, out=d["y2"][:, :], in0=d["w"][:, :], in1=d["y1"][:, :],
                                      s0=NRW_C0, s1=NRW_C1, imm2=NRW_C2)
                g_mul(trhs[l][0:1, :], d["sx"][:, :], d["y2"][:, :])
                for m in range(2):
                    lq = (2 * l + m) * 128
                    nc.tensor.matmul(tl[:, m * 64:(m + 1) * 64], W["lngN"][:, lq:lq + 128],
                                     d["y2"][0:1, :], start=True, stop=True, skip_group_check=True)
                    nc.tensor.matmul(tl[:, 128 + m * 64:128 + (m + 1) * 64], W["lnT"][:, lq:lq + 128],
                                     trhs[l][:, :], start=True, stop=True, skip_group_check=True)
                nc.vector.scalar_tensor_tensor(out=d["z"][:, :], in0=d["P"][:, :], scalar=0.0,
                                               in1=tl[:, 0:128], op0=OP.add, op1=OP.mult)
                nc.vector.scalar_tensor_tensor(out=d["z2"][:, :], in0=d["z"][:, :], scalar=0.0,
                                               in1=tl[:, 128:256], op0=OP.add, op1=OP.add)
                nc.scalar.activation(h_next[:, :], d["z2"][:, :], AF.Tanh)

            r0, r1 = rings[0], rings[1]

            def a_phases(tau, xc):
                """A = layer0 tick tau: [s1, s2, s3, s4, tail_a, tail_b]."""
                hs = hz if tau == 0 else r0[(tau - 1) % 4]
                return [
                    lambda: emit_stage(0, 1, xc=xc, h_self=hs),
                    lambda: emit_stage(0, 2, h_self=hs),
                    lambda: emit_stage(0, 3, h_self=hs),
                    lambda: emit_stage(0, 4, h_self=hs),
                    lambda: emit_tail_a(0, hs),
                    lambda: emit_tail_b(0, r0[tau % 4]),
                ]

            def b_phases(tau):
                """B = layer1 tick tau-1 (emitted during slots tau/tau+1).
                For tau==1, r1[3] is pre-zeroed and serves as h1[-1]."""
                hs = r1[(tau - 2) % 4]
                x1 = r0[(tau - 1) % 4]
                return [
                    lambda: emit_stage(1, 1, x1=x1, h_self=hs),
                    lambda: emit_stage(1, 2, h_self=hs),
                    lambda: emit_stage(1, 3, h_self=hs),
                    lambda: emit_stage(1, 4, h_self=hs),
                    lambda: emit_tail_a(1, hs),
                    lambda: emit_tail_b(1, r1[(tau - 1) % 4]),
                ]

            def emit_slot(a_ph, b_prev, b_cur):
                """Steady-state slot: A's 6 phases; B(prev slot) finishes its
                last 3 phases under A's first stages, B(cur) starts its first
                3 under A's tail."""
                order = []
                if a_ph:
                    order.append(a_ph[0])
                if b_prev:
                    order.append(b_prev[3])
                if a_ph:
                    order.append(a_ph[1])
                if b_prev:
                    order.append(b_prev[4])
                if a_ph:
                    order.append(a_ph[2])
                if b_prev:
                    order.append(b_prev[5])
                if a_ph:
                    order.append(a_ph[3])
                if b_cur:
                    order.append(b_cur[0])
                if a_ph:
                    order.append(a_ph[4])
                if b_cur:
                    order.append(b_cur[1])
                if a_ph:
                    order.append(a_ph[5])
                if b_cur:
                    order.append(b_cur[2])
                for f in order:
                    f()

            # ---- head: slots 0..1 ----------------------------------------
            xc0 = xc_pool.tile([97, BC], bf16, tag="xc")
            nc.sync.dma_start(xc0[:], xcat[0])
            emit_slot(a_phases(0, xc0), None, None)
            xc1 = xc_pool.tile([97, BC], bf16, tag="xc")
            nc.sync.dma_start(xc1[:], xcat[1])
            emit_slot(a_phases(1, xc1), None, b_phases(1))

            # ---- main loop: slots 2..505 ---------------------------------
            with tc.For_i(LOOP_LO, LOOP_HI, unroll) as iv:
                xslab = xcat[ds(iv, unroll)]
                xcu = []
                for u in range(unroll):
                    t_ = xc_pool.tile([97, BC], bf16, tag="xc")
                    nc.sync.dma_start(t_[:], xslab[u])
                    xcu.append(t_)
                for u in range(unroll):
                    tau = LOOP_LO + u  # slot = iv+u; mod-4 matches since iv%8==2
                    emit_slot(a_phases(tau, xcu[u]), b_phases(tau - 1), b_phases(tau))

            # ---- tail: slots 506..513 ------------------------------------
            for tau in range(LOOP_HI, T_run + 2):
                do_a = tau < T_run
                if do_a:
                    xct = xc_pool.tile([97, BC], bf16, tag="xc")
                    nc.sync.dma_start(xct[:], xcat[tau])
                    ap = a_phases(tau, xct)
                else:
                    ap = None
                bp_prev = b_phases(tau - 1) if tau - 1 <= T_run else None
                bp_cur = b_phases(tau) if tau <= T_run else None
                emit_slot(ap, bp_prev, bp_cur)

            nc.sync.dma_start(hout[:], rings[1][(T_run - 1) % 4][:])

    nc.compile()
    return nc


# ---------------------------------------------------------------------------
# Host-side weight prep
# ---------------------------------------------------------------------------

def _prep_weights(inputs):
    import ml_dtypes
    bf = ml_dtypes.bfloat16
    W = {k: np.asarray(v, np.float32) for k, v in inputs.items()}
    out = {}

    def pack_kt(wT):  # [256, 256] -> [128, 512] (kt, m)
        return np.concatenate([wT[0:128, :], wT[128:256, :]], axis=1)

    for l in range(L):
        fin = FS + FC if l == 0 else H
        Wg, Win, Wrec = W[f'Wg{l}'], W[f'Win{l}'], W[f'Wrec{l}']
        bg, tau = W[f'bg{l}'], W[f'tau{l}']
        itau = (1.0 / (softplus_np(tau) + 1.0)).astype(np.float32)
        WgxT = Wg[:, :fin].T
        WghT = Wg[:, fin:].T
        WrecT = Wrec.T
        WinT = Win.T
        dn = np.zeros((128, 256), np.float32)
        dnh = np.zeros((128, 256), np.float32)
        for m in range(2):
            dn[:, m * 128:(m + 1) * 128] = np.diag(-itau[m * 128:(m + 1) * 128])
            dnh[:, m * 128:(m + 1) * 128] = np.diag(-0.5 * itau[m * 128:(m + 1) * 128])
        if l == 0:
            out["g_x0"] = np.concatenate([WgxT, bg[None, :]], 0).astype(bf)
            out["in_x0"] = WinT.astype(bf)
            out["g_h0"] = pack_kt(WghT).astype(bf)
            out["g_h0h"] = pack_kt(WghT * 0.5).astype(bf)
            out["rec0"] = pack_kt(WrecT).astype(bf)
            out["rec0h"] = pack_kt(WrecT * 0.5).astype(bf)
            out["dneg0"] = dn.astype(bf)
            out["dneg0h"] = dnh.astype(bf)
        else:
            out["g_x1"] = pack_kt(WgxT).astype(bf)
            out["bg1row"] = bg[None, :].astype(bf)
            out["in_x1"] = pack_kt(WinT).astype(bf)
            out["g_h1"] = pack_kt(WghT).astype(bf)
            out["g_h1h"] = pack_kt(WghT * 0.5).astype(bf)
            out["rec1"] = pack_kt(WrecT).astype(bf)
            out["rec1h"] = pack_kt(WrecT * 0.5).astype(bf)
            out["dneg1"] = dn.astype(bf)
            out["dneg1h"] = dnh.astype(bf)
    lnT = np.zeros((2, 512), np.float32)
    lngN = np.zeros((1, 512), np.float32)
    for l in range(L):
        lng, lnb = W[f'lng{l}'], W[f'lnb{l}']
        for m in range(2):
            lq = (2 * l + m) * 128
            lnT[0, lq:lq + 128] = -lng[m * 128:(m + 1) * 128]
            lnT[1, lq:lq + 128] = lnb[m * 128:(m + 1) * 128]
            lngN[0, lq:lq + 128] = lng[m * 128:(m + 1) * 128] * N_H
    out["lnT"] = lnT.astype(bf)
    out["lngN"] = lngN.astype(bf)
    out["ones_red"] = np.ones((128, 1), np.float32).astype(bf)
    out["ones2"] = np.ones((2, 64), np.float32).astype(bf)
    return out


def _prep_core_inputs(inputs, wpack, core, T_run=T):
    seq = np.asarray(inputs['seq_features'], np.float32)
    ctx = np.asarray(inputs['context_features'], np.float32)
    bsl = slice(core * BC, (core + 1) * BC)
    import ml_dtypes
    xc = np.empty((T_run, 97, BC), np.float32)
    xc[:, 0:64, :] = seq[bsl, :T_run].transpose(1, 2, 0)
    xc[:, 64:96, :] = ctx[bsl].T[None, :, :]
    xc[:, 96, :] = 1.0
    m = {"xcat": xc.astype(ml_dtypes.bfloat16)}
    m.update(wpack)
    return m


def _head(inputs, h1):
    cW1 = np.asarray(inputs['cW1'], np.float32)
    cb1 = np.asarray(inputs['cb1'], np.float32)
    cW2 = np.asarray(inputs['cW2'], np.float32)
    cb2 = np.asarray(inputs['cb2'], np.float32)
    hid = np.maximum(h1 @ cW1.T + cb1, 0)
    return (hid @ cW2.T + cb2).squeeze(-1)


_CACHE = {}


def kernel(**inputs):
    if "nc" not in _CACHE:
        _CACHE["nc"] = build_module(T, UNROLL)
    nc = _CACHE["nc"]
    from concourse.bass_utils import run_bass_kernel_spmd
    wpack = _prep_weights(inputs)
    in_maps = [_prep_core_inputs(inputs, wpack, c) for c in range(NCORES)]
    do_trace = os.environ.get("BASS_KERNEL_TRACE") == "1"
    r = run_bass_kernel_spmd(nc, in_maps, list(range(NCORES)), trace=do_trace)
    res = r.results
    if do_trace:
        _CACHE["exec_ns"] = r.exec_time_ns
        if r.instructions_and_trace is not None:
            _CACHE["trace_path"] = r.instructions_and_trace[1]
    h1 = np.empty((B, H), np.float32)
    for c in range(NCORES):
        ht = np.asarray(res[c]["hout"], np.float32)  # [128, (m,b)]
        bsl = slice(c * BC, (c + 1) * BC)
        for m in range(2):
            h1[bsl, m * 128:(m + 1) * 128] = ht[:, m * 64:(m + 1) * 64].T
    return _head(inputs, h1).astype(np.float32)


if __name__ == "__main__":
    pass
